# revision 1
# baseline (speedup 1.0000x reference)
"""CenterWeightedCIoULoss on 8 Trainium2 NeuronCores (Bass/Tile).

Math (per matched box pair, xyxy boxes):
    loss = (1 - iou) + 2*center_term + size_term,  output = mean(loss)

Rescaled identities used on-chip (exact, incl. the reference's eps placement):
    d1 = p1 - t1, d2 = p2 - t2           (per coord)
    u  = |d1| + |d2|,  a = pwh + twh     (pwh = p2-p1, twh = t2-t1)
    2*inter_wh = relu(a - u),  2*c_wh = a + u,  2*(pc-tc) = d1 + d2
    iou          = inter4 / (4*(p_area + t_area + eps) - inter4)
    center_term  = cdsq4 / (cwh2x^2 + cwh2y^2 + 8*eps)
    size_term    = (pw/tw - 1)^2 + (ph/th - 1)^2
Host adds the constant 1 (from 1 - iou) after the mean.

Reciprocals run on the Scalar engine as exp(-ln(x)) (one table set holds
ln/exp/relu/abs/square); the Vector engine keeps only 2-input arithmetic.
Sharding: boxes split evenly over 8 cores; each core reduces its shard to
[128, 3*T] partial sums which the host sums in f64.
"""

import sys

sys.path.insert(0, "/opt/trn_rl_repo")

import numpy as np

import concourse.bass as bass
import concourse.bacc as bacc
import concourse.tile as tile
from concourse import mybir
from concourse.bass_utils import run_bass_kernel_spmd

N = 4_194_304
NCORES = 8
NB = N // NCORES            # boxes per core
P = 128
PERPART = NB * 4 // P       # 16384 f32 per partition per tensor
CHUNK = 2048                # f32 per partition per tile
T = PERPART // CHUNK        # 8 tiles
BX = CHUNK // 4             # boxes per partition per tile
EPS = 1e-7

F32 = mybir.dt.float32
Alu = mybir.AluOpType
Act = mybir.ActivationFunctionType

D_ON_GPSIMD = True

# All activation funcs used here (Abs/Ln/Exp/Relu/Square/Identity) live in
# the single 'natural_log_exp_and_others' table set, but bacc's greedy
# per-instruction set chooser bounces between sets (4 table loads per tile,
# ~2.7us each). Restrict the candidate tables to that one set (other
# entries kept, emptied, to preserve act_func_set_id indices).
_orig_get_tables = bacc.get_activation_tables


def _pinned_tables(arch):
    tables = _orig_get_tables(arch)
    pinned = "natural_log_exp_and_others"
    assert pinned in tables
    return {
        name: (funcs if name == pinned else set())
        for name, funcs in tables.items()
    }


bacc.get_activation_tables = _pinned_tables

_compiled = None


def _build():
    nc = bacc.Bacc("TRN2", target_bir_lowering=False, debug=False)
    pred = nc.dram_tensor("pred", [NB, 4], F32, kind="ExternalInput").ap()
    targ = nc.dram_tensor("targ", [NB, 4], F32, kind="ExternalInput").ap()
    out = nc.dram_tensor("out", [P, 3 * T], F32, kind="ExternalOutput").ap()

    predv = pred.rearrange("(p n) c -> p (n c)", p=P)
    targv = targ.rearrange("(p n) c -> p (n c)", p=P)

    with tile.TileContext(nc) as tc:
        with (
            tc.tile_pool(name="io", bufs=3) as io,
            tc.tile_pool(name="mid", bufs=2) as mid,
            tc.tile_pool(name="acc", bufs=1) as accp,
        ):
            acc = accp.tile([P, 3 * T], F32)
            for t in range(T):
                sl = slice(t * CHUNK, (t + 1) * CHUNK)
                Pt = io.tile([P, CHUNK], F32, tag="p")
                Tt = io.tile([P, CHUNK], F32, tag="t")
                nc.sync.dma_start(Pt[:], predv[:, sl])
                nc.sync.dma_start(Tt[:], targv[:, sl])

                P4 = Pt[:].rearrange("p (n c) -> p n c", c=4)
                T4 = Tt[:].rearrange("p (n c) -> p n c", c=4)

                # D = P - T (all 4 coords), contiguous 2-input.
                D = mid.tile([P, CHUNK], F32, tag="D")
                if D_ON_GPSIMD:
                    nc.gpsimd.tensor_tensor(D[:], Pt[:], Tt[:], Alu.subtract)
                else:
                    nc.vector.tensor_sub(D[:], Pt[:], Tt[:])
                D4 = D[:].rearrange("p (n c) -> p n c", c=4)

                def half(x):  # [P, BX, 2] view of a [P, 2*BX] tile
                    return x[:].rearrange("p (n c) -> p n c", c=2)

                pwh = mid.tile([P, 2 * BX], F32, tag="pwh")
                twh = mid.tile([P, 2 * BX], F32, tag="twh")
                nc.vector.tensor_sub(half(pwh), P4[:, :, 2:4], P4[:, :, 0:2])
                nc.vector.tensor_sub(half(twh), T4[:, :, 2:4], T4[:, :, 0:2])

                a = mid.tile([P, 2 * BX], F32, tag="a")
                nc.gpsimd.tensor_tensor(a[:], pwh[:], twh[:], Alu.add)

                B = mid.tile([P, CHUNK], F32, tag="B")
                nc.scalar.activation(B[:], D[:], Act.Abs)
                B4 = B[:].rearrange("p (n c) -> p n c", c=4)

                u = mid.tile([P, 2 * BX], F32, tag="u")
                nc.vector.tensor_add(half(u), B4[:, :, 0:2], B4[:, :, 2:4])
                cd2 = mid.tile([P, 2 * BX], F32, tag="cd2")
                nc.vector.tensor_add(half(cd2), D4[:, :, 0:2], D4[:, :, 2:4])

                s = mid.tile([P, 2 * BX], F32, tag="s")
                nc.vector.tensor_sub(s[:], a[:], u[:])
                cwh2 = mid.tile([P, 2 * BX], F32, tag="cwh2")
                nc.gpsimd.tensor_tensor(cwh2[:], a[:], u[:], Alu.add)

                # rtw = 1/twh as exp(-ln(twh)); both stages on ScalarE,
                # exp in place over the ln result.
                rtw = mid.tile([P, 2 * BX], F32, tag="rtw")
                nc.scalar.activation(rtw[:], twh[:], Act.Ln)
                nc.scalar.activation(rtw[:], rtw[:], Act.Exp, scale=-1.0)
                q1 = mid.tile([P, 2 * BX], F32, tag="q1")
                nc.vector.tensor_mul(q1[:], pwh[:], rtw[:])

                # In-place ACT stages: relu/squares overwrite their inputs.
                nc.scalar.activation(s[:], s[:], Act.Relu)              # iw2
                nc.scalar.activation(cd2[:], cd2[:], Act.Square)        # sqcd
                nc.scalar.activation(cwh2[:], cwh2[:], Act.Square)      # sqcw
                # size_term elements (1 - q1)^2 == (q1 - 1)^2, accumulated;
                # scratch output lands on the dead `a` tile.
                nc.scalar.activation(
                    a[:], q1[:], Act.Square, bias=1.0, scale=-1.0,
                    accum_out=acc[:, 3 * t + 2 : 3 * t + 3],
                )

                def xy(x):  # x,y component views [P, BX]
                    v = x[:].rearrange("p (n c) -> p n c", c=2)
                    return v[:, :, 0], v[:, :, 1]

                iw2x, iw2y = xy(s)
                inter4 = mid.tile([P, BX], F32, tag="inter4")
                nc.vector.tensor_mul(inter4[:], iw2x, iw2y)
                pwx, pwy = xy(pwh)
                ap_ = mid.tile([P, BX], F32, tag="ap")
                nc.vector.tensor_mul(ap_[:], pwx, pwy)
                twx, twy = xy(twh)
                at_ = mid.tile([P, BX], F32, tag="at")
                nc.vector.tensor_mul(at_[:], twx, twy)
                sa4 = mid.tile([P, BX], F32, tag="sa4")
                nc.vector.affine_then_add(sa4[:], ap_[:], at_[:], 1.0, EPS)
                union4 = mid.tile([P, BX], F32, tag="union4")
                nc.vector.scalar_tensor_tensor(
                    union4[:], sa4[:], 4.0, inter4[:], Alu.mult, Alu.subtract
                )
                # ru = 1/union4 in place (ln then exp(-x)).
                nc.scalar.activation(union4[:], union4[:], Act.Ln)
                nc.scalar.activation(union4[:], union4[:], Act.Exp, scale=-1.0)
                nc.vector.affine_mul_reduce(
                    ap_[:], acc[:, 3 * t : 3 * t + 1], inter4[:], union4[:],
                    -1.0, 0.0,
                )

                sqcdx, sqcdy = xy(cd2)
                cdsq4 = mid.tile([P, BX], F32, tag="cdsq4")
                nc.vector.tensor_add(cdsq4[:], sqcdx, sqcdy)
                sqcwx, sqcwy = xy(cwh2)
                cdiag4 = mid.tile([P, BX], F32, tag="cdiag4")
                nc.vector.affine_then_add(cdiag4[:], sqcwx, sqcwy, 1.0, 8 * EPS)
                # rc = 1/cdiag4 in place.
                nc.scalar.activation(cdiag4[:], cdiag4[:], Act.Ln)
                nc.scalar.activation(cdiag4[:], cdiag4[:], Act.Exp, scale=-1.0)
                nc.vector.affine_mul_reduce(
                    at_[:], acc[:, 3 * t + 1 : 3 * t + 2], cdsq4[:], cdiag4[:],
                    2.0, 0.0,
                )
            nc.sync.dma_start(out[:], acc[:])
    nc.compile()
    return nc


def kernel(pred_boxes: np.ndarray, target_boxes: np.ndarray) -> np.ndarray:
    global _compiled
    if _compiled is None:
        _compiled = _build()
    nc = _compiled
    preds = np.split(np.ascontiguousarray(pred_boxes, np.float32), NCORES, axis=0)
    targs = np.split(np.ascontiguousarray(target_boxes, np.float32), NCORES, axis=0)
    in_maps = [{"pred": preds[i], "targ": targs[i]} for i in range(NCORES)]
    res = run_bass_kernel_spmd(nc, in_maps, core_ids=list(range(NCORES))).results
    total = 0.0
    for r in res:
        total += np.sum(r["out"].astype(np.float64))
    return np.float32(1.0 + total / N)

